# revision 10
# baseline (speedup 1.0000x reference)
"""Distributed multi-head attention kernel for 8 TRN2 NeuronCores.

Problem: x[4,2048,1024] -> qkv proj (w_qkv[3072,1024]) -> 16-head attention
         -> out proj (w_out[1024,1024], b_out) -> [4,2048,1024]

Sharding: core c handles batch b=c//2 and heads (c%2)*8 .. (c%2)*8+8
(data parallel over batch x tensor parallel over heads). Each pair of
cores {2b, 2b+1} reduce-scatters the output projection partial sums in
bf16, one collective per 512-token chunk (a single RS per chunk: each
RS carries ~5us fixed cost so fewer+bigger wins).

Per-core compute (bf16 matmuls, fp32 PSUM):
  The exp stream on ScalarE is the throughput bottleneck (~1.2us per
  (pr, jt) unit vs ~0.64us of PE), so the schedule starts it as early
  as possible: only K m0 + Q m0 + V j0/j1 run before the first scores,
  and all remaining projection work (K m1-m3, Q, V j2-j15, prior
  chunk's out-proj) is spliced into the attention stream as "filler"
  closures, one per unit, emitted just before each PV so the in-order
  PE queue executes them while ScalarE computes the unit's exp.

  Scores are computed transposed, S^T [j, i], two heads packed in PE
  row groups 0/64. exp runs on ScalarE with the 1/sqrt(d) scale folded
  in. Softmax denominators come from ones-matmul column sums packed in
  PE col groups 0/64. O^T accumulates in PSUM (V stationary); VectorE
  applies a fast reciprocal and normalizes into mergedT (bf16), which
  feeds the output projection as the moving operand.

  PSUM budget (8 banks): st 2x2 + po 1 + pd 1 + proj 2.
"""

import numpy as np
import ml_dtypes

import concourse.bass as bass
import concourse.mybir as mybir
import concourse.tile as tile
from concourse import bacc
from concourse.bass_utils import run_bass_kernel_spmd

B, N, H = 4, 2048, 1024
NH, DH = 16, 64
NCORES = 8
HH = 512          # head dims per core (8 heads x 64)
KH = H // 128     # 8 hidden k-tiles
NJT = N // 128    # 16 token j-tiles
NCK = N // 512    # 4 token chunks
MT = HH // 128    # 4 head-dim partition tiles per core
SCALE = DH ** -0.5

BF16 = mybir.dt.bfloat16
F32 = mybir.dt.float32
Exp = mybir.ActivationFunctionType.Exp

RG = [[0, 1], [2, 3], [4, 5], [6, 7]]

_cache = {}


def _build():
    nc = bacc.Bacc(
        "TRN2", target_bir_lowering=False, debug=False, num_devices=NCORES
    )
    xT = nc.dram_tensor("xT", [H, N], BF16, kind="ExternalInput").ap()
    wqT = nc.dram_tensor("wqT", [H, HH], BF16, kind="ExternalInput").ap()
    wkT = nc.dram_tensor("wkT", [H, HH], BF16, kind="ExternalInput").ap()
    wvT = nc.dram_tensor("wvT", [H, HH], BF16, kind="ExternalInput").ap()
    woT = nc.dram_tensor("woT", [HH, H], BF16, kind="ExternalInput").ap()
    hbT = nc.dram_tensor("hbT", [128, KH], F32, kind="ExternalInput").ap()
    out_e = nc.dram_tensor("out", [H // 2, N], BF16, kind="ExternalOutput").ap()

    with tile.TileContext(nc) as tc:
        with (
            tc.tile_pool(name="singles", bufs=1) as singles,
            tc.tile_pool(name="ps", bufs=1, space="PSUM") as ps,
            tc.tile_pool(name="pe", bufs=16) as pe_pool,
            tc.tile_pool(name="rsb", bufs=6) as r_pool,
            tc.tile_pool(name="osb", bufs=6) as osb_pool,
            tc.tile_pool(name="dram", bufs=1, space="DRAM") as dram,
        ):
            x_sb = singles.tile([128, KH, N], BF16)
            wq_sb = singles.tile([128, KH, HH], BF16)
            wk_sb = singles.tile([128, KH, HH], BF16)
            wv_sb = singles.tile([128, KH, HH], BF16)
            wo_sb = singles.tile([128, MT, H], BF16)
            hb_sb = singles.tile([128, KH], F32)
            qT_sb = singles.tile([128, MT, N], BF16)
            kT_sb = singles.tile([128, MT, N], BF16)
            v_sb = singles.tile([128, NJT, 8, DH], BF16)
            mT_sb = singles.tile([128, MT, N], BF16)
            ones_sb = singles.tile([128, DH], BF16)

            rs_in = []
            rs_out = []
            for c in range(NCK):
                t_in = dram.tile([H, 512], BF16, tag=f"rsin{c}", name=f"rsin{c}")
                t_out = dram.tile([H // 2, 512], BF16, tag=f"rsout{c}",
                                  name=f"rsout{c}")
                rs_in.append(t_in)
                rs_out.append(t_out)

            nc.vector.memset(ones_sb, 1.0)
            # three DMA issue queues so the critical inputs (x halves, wk,
            # wq, wv) all land within the first ~12us
            for k in range(KH):
                nc.gpsimd.dma_start(out=x_sb[:, k, 0:1024],
                                    in_=xT[k * 128:(k + 1) * 128, 0:1024])
            for k in range(KH):
                nc.gpsimd.dma_start(out=x_sb[:, k, 1024:2048],
                                    in_=xT[k * 128:(k + 1) * 128, 1024:2048])
            for k in range(KH):
                nc.sync.dma_start(out=wk_sb[:, k, :], in_=wkT[k * 128:(k + 1) * 128, :])
            for k in range(KH):
                nc.sync.dma_start(out=wq_sb[:, k, :], in_=wqT[k * 128:(k + 1) * 128, :])
            for k in range(KH):
                nc.scalar.dma_start(out=wv_sb[:, k, :], in_=wvT[k * 128:(k + 1) * 128, :])
            for m in range(MT):
                nc.scalar.dma_start(out=wo_sb[:, m, :], in_=woT[m * 128:(m + 1) * 128, :])
            nc.scalar.dma_start(out=hb_sb, in_=hbT)

            def kq_proj(w_sb, dst, m, ci):
                pp = ps.tile([128, 512], F32, tag="proj", bufs=2,
                             name=f"kq{m}_{ci}")
                for k in range(KH):
                    nc.tensor.matmul(
                        pp,
                        lhsT=w_sb[:, k, m * 128:(m + 1) * 128],
                        rhs=x_sb[:, k, ci * 512:(ci + 1) * 512],
                        start=(k == 0), stop=(k == KH - 1),
                    )
                nc.vector.tensor_copy(out=dst[:, m, ci * 512:(ci + 1) * 512], in_=pp)

            def v_proj(jt):
                pp = ps.tile([128, 512], F32, tag="proj", bufs=2,
                             name=f"vp{jt}")
                for k in range(KH):
                    nc.tensor.matmul(
                        pp,
                        lhsT=x_sb[:, k, jt * 128:(jt + 1) * 128],
                        rhs=wv_sb[:, k, :],
                        start=(k == 0), stop=(k == KH - 1),
                    )
                nc.vector.tensor_copy(
                    out=v_sb[:, jt, :, :],
                    in_=pp.rearrange("p (h d) -> p h d", h=8),
                )

            def outproj_m(ci, m):
                pp = ps.tile([128, 512], F32, tag="proj", bufs=2,
                             name=f"pp{ci}_{m}")
                for kd in range(MT):
                    nc.tensor.matmul(
                        pp,
                        lhsT=wo_sb[:, kd, m * 128:(m + 1) * 128],
                        rhs=mT_sb[:, kd, ci * 512:(ci + 1) * 512],
                        start=(kd == 0), stop=(kd == MT - 1),
                    )
                ob = osb_pool.tile([128, 512], BF16, tag="ob")
                nc.vector.tensor_scalar_add(
                    out=ob, in0=pp, scalar1=hb_sb[:, m:m + 1]
                )
                nc.sync.dma_start(
                    out=rs_in[ci][m * 128:(m + 1) * 128, :], in_=ob
                )
                if m == 7:
                    nc.gpsimd.collective_compute(
                        "ReduceScatter",
                        mybir.AluOpType.add,
                        replica_groups=RG,
                        ins=[rs_in[ci][:, :]],
                        outs=[rs_out[ci][:, :]],
                    )
                    nc.sync.dma_start(
                        out=out_e[:, ci * 512:(ci + 1) * 512],
                        in_=rs_out[ci][:, :],
                    )

            def attention_ci(ci, fillers):
                units = [(pr, jt) for pr in range(4) for jt in range(NJT)]
                st_tiles = {}

                def emit_scores(u):
                    pr, jt = units[u]
                    st = ps.tile([128, 2, 512], F32, tag="st", bufs=2,
                                 name=f"st{ci}_{u}")
                    nc.tensor.matmul(
                        st[:, 0, :],
                        lhsT=kT_sb[0:64, pr, jt * 128:(jt + 1) * 128],
                        rhs=qT_sb[0:64, pr, ci * 512:(ci + 1) * 512],
                        start=True, stop=True,
                    )
                    nc.tensor.matmul(
                        st[:, 1, :],
                        lhsT=kT_sb[64:128, pr, jt * 128:(jt + 1) * 128],
                        rhs=qT_sb[64:128, pr, ci * 512:(ci + 1) * 512],
                        start=True, stop=True,
                    )
                    st_tiles[u] = st

                def emit_pv(po, pd, pe, pr, jt, first, last):
                    h0, h1 = 2 * pr, 2 * pr + 1
                    nc.tensor.matmul(
                        po[0:64, :], lhsT=v_sb[:, jt, h0, :], rhs=pe[:, 0, :],
                        start=first, stop=last, skip_group_check=True,
                    )
                    nc.tensor.matmul(
                        po[64:128, :], lhsT=v_sb[:, jt, h1, :], rhs=pe[:, 1, :],
                        start=first, stop=last, skip_group_check=True,
                    )
                    nc.tensor.matmul(
                        pd[0:64, :], lhsT=ones_sb, rhs=pe[:, 0, :],
                        start=first, stop=last, skip_group_check=True,
                    )
                    nc.tensor.matmul(
                        pd[64:128, :], lhsT=ones_sb, rhs=pe[:, 1, :],
                        start=first, stop=last, skip_group_check=True,
                    )

                emit_scores(0)
                emit_scores(1)
                po = pd = None
                fi = 0
                for u, (pr, jt) in enumerate(units):
                    first, last = (jt == 0), (jt == NJT - 1)
                    if first:
                        po = ps.tile([128, 512], F32, tag="po", bufs=1,
                                     name=f"po{pr}")
                        pd = ps.tile([128, 512], F32, tag="pd", bufs=1,
                                     name=f"pd{pr}")
                    pe = pe_pool.tile([128, 2, 512], BF16, tag="pe")
                    nc.scalar.activation(out=pe, in_=st_tiles.pop(u),
                                         func=Exp, scale=SCALE)
                    if u + 2 < len(units):
                        emit_scores(u + 2)
                    if fi < len(fillers):
                        fillers[fi]()
                        fi += 1
                    emit_pv(po, pd, pe, pr, jt, first, last)
                    if last:
                        r = r_pool.tile([128, 512], F32, tag="r")
                        nc.vector.reciprocal_approx_fast(out=r, in_=pd)
                        nc.vector.tensor_mul(
                            out=mT_sb[:, pr, ci * 512:(ci + 1) * 512],
                            in0=po, in1=r,
                        )
                while fi < len(fillers):
                    fillers[fi]()
                    fi += 1

            def F(fn, *a):
                return lambda: fn(*a)

            # ---- prologue: just enough for the exp stream to start
            for ci in range(NCK):
                kq_proj(wk_sb, kT_sb, 0, ci)
            kq_proj(wq_sb, qT_sb, 0, 0)
            v_proj(0)
            v_proj(1)

            # ---- chunk 0: splice the rest of K/Q/V as fillers
            fillers0 = [F(kq_proj, wk_sb, kT_sb, 1, 0), F(kq_proj, wq_sb, qT_sb, 1, 0)]
            fillers0 += [F(v_proj, jt) for jt in range(2, NJT)]
            fillers0 += [F(kq_proj, wk_sb, kT_sb, 1, ci) for ci in (1, 2, 3)]
            fillers0 += [F(kq_proj, wk_sb, kT_sb, 2, 0), F(kq_proj, wq_sb, qT_sb, 2, 0)]
            fillers0 += [F(kq_proj, wk_sb, kT_sb, 2, ci) for ci in (1, 2, 3)]
            fillers0 += [F(kq_proj, wk_sb, kT_sb, 3, 0), F(kq_proj, wq_sb, qT_sb, 3, 0)]
            fillers0 += [F(kq_proj, wk_sb, kT_sb, 3, ci) for ci in (1, 2, 3)]
            fillers0 += [F(kq_proj, wq_sb, qT_sb, m, 1) for m in range(MT)]
            attention_ci(0, fillers0)

            # ---- chunks 1..3: splice prior chunk's out-proj + next Q
            for ci in range(1, NCK):
                fillers = [F(outproj_m, ci - 1, m) for m in range(8)]
                if ci + 1 < NCK:
                    fillers += [F(kq_proj, wq_sb, qT_sb, m, ci + 1)
                                for m in range(MT)]
                attention_ci(ci, fillers)
            for m in range(8):
                outproj_m(NCK - 1, m)

    nc.compile()
    return nc


def _get_nc():
    if "nc" not in _cache:
        _cache["nc"] = _build()
    return _cache["nc"]


def _shard_inputs(x, w_qkv, w_out, b_out):
    bf16 = ml_dtypes.bfloat16
    in_maps = []
    for c in range(NCORES):
        b, hh = c // 2, c % 2
        r0 = hh * HH
        hbT = (0.5 * b_out).astype(np.float32).reshape(KH, 128).T
        in_maps.append({
            "xT": np.ascontiguousarray(x[b].T).astype(bf16),
            "wqT": np.ascontiguousarray(w_qkv[r0:r0 + HH, :].T).astype(bf16),
            "wkT": np.ascontiguousarray(w_qkv[H + r0:H + r0 + HH, :].T).astype(bf16),
            "wvT": np.ascontiguousarray(w_qkv[2 * H + r0:2 * H + r0 + HH, :].T).astype(bf16),
            "woT": np.ascontiguousarray(w_out[:, r0:r0 + HH].T).astype(bf16),
            "hbT": np.ascontiguousarray(hbT),
        })
    return in_maps


def _assemble(results):
    out = np.empty((B, N, H), dtype=np.float32)
    for b in range(B):
        lo = np.asarray(results[2 * b]["out"]).astype(np.float32)
        hi = np.asarray(results[2 * b + 1]["out"]).astype(np.float32)
        out[b][:, :512] = lo.T
        out[b][:, 512:] = hi.T
    return out


def run_sharded(x, w_qkv, w_out, b_out, trace=False):
    nc = _get_nc()
    in_maps = _shard_inputs(x, w_qkv, w_out, b_out)
    res = run_bass_kernel_spmd(nc, in_maps, core_ids=list(range(NCORES)),
                               trace=trace)
    return _assemble(res.results), res


def kernel(x, w_qkv, w_out, b_out):
    x = np.asarray(x, dtype=np.float32)
    w_qkv = np.asarray(w_qkv, dtype=np.float32)
    w_out = np.asarray(w_out, dtype=np.float32)
    b_out = np.asarray(b_out, dtype=np.float32)
    out, _ = run_sharded(x, w_qkv, w_out, b_out, trace=False)
    return out


# revision 15
# speedup vs baseline: 1.0232x; 1.0232x over previous
"""Distributed multi-head attention kernel for 8 TRN2 NeuronCores.

Problem: x[4,2048,1024] -> qkv proj (w_qkv[3072,1024]) -> 16-head attention
         -> out proj (w_out[1024,1024], b_out) -> [4,2048,1024]

Sharding: core c handles batch b=c//2 and heads (c%2)*8 .. (c%2)*8+8
(data parallel over batch x tensor parallel over heads). Each pair of
cores {2b, 2b+1} reduce-scatters the output projection partial sums in
bf16, one collective per 512-token chunk (a single RS per chunk: each
RS carries ~5us fixed cost so fewer+bigger wins).

Per-core compute (bf16 matmuls, fp32 PSUM):
  The exp stream on ScalarE is the throughput bottleneck (~1.2us per
  (pr, jt) unit vs ~0.64us of PE), so the schedule starts it as early
  as possible: only K m0 + Q m0 + V j0/j1 run before the first scores,
  and all remaining projection work (K m1-m3, Q, V j2-j15, prior
  chunk's out-proj) is spliced into the attention stream as "filler"
  closures, one per unit, emitted just before each PV so the in-order
  PE queue executes them while ScalarE computes the unit's exp.

  Scores are computed transposed, S^T [j, i], two heads packed in PE
  row groups 0/64. exp runs on ScalarE with the 1/sqrt(d) scale folded
  in. Softmax denominators come from ones-matmul column sums packed in
  PE col groups 0/64. O^T accumulates in PSUM (V stationary); VectorE
  applies a fast reciprocal and normalizes into mergedT (bf16), which
  feeds the output projection as the moving operand.

  PSUM budget (8 banks): st 2x2 + po 1 + pd 1 + proj 2.
"""

import numpy as np
import ml_dtypes

import concourse.bass as bass
import concourse.mybir as mybir
import concourse.tile as tile
from concourse import bacc
from concourse.bass_utils import run_bass_kernel_spmd

B, N, H = 4, 2048, 1024
NH, DH = 16, 64
NCORES = 8
HH = 512          # head dims per core (8 heads x 64)
KH = H // 128     # 8 hidden k-tiles
NJT = N // 128    # 16 token j-tiles
NCK = N // 512    # 4 token chunks
MT = HH // 128    # 4 head-dim partition tiles per core
SCALE = DH ** -0.5

BF16 = mybir.dt.bfloat16
F32 = mybir.dt.float32
Exp = mybir.ActivationFunctionType.Exp

RG = [[0, 1], [2, 3], [4, 5], [6, 7]]

_cache = {}


def _build():
    nc = bacc.Bacc(
        "TRN2", target_bir_lowering=False, debug=False, num_devices=NCORES
    )
    xT = nc.dram_tensor("xT", [H, N], BF16, kind="ExternalInput").ap()
    wqT = nc.dram_tensor("wqT", [H, HH], BF16, kind="ExternalInput").ap()
    wkT = nc.dram_tensor("wkT", [H, HH], BF16, kind="ExternalInput").ap()
    wvT = nc.dram_tensor("wvT", [H, HH], BF16, kind="ExternalInput").ap()
    woT = nc.dram_tensor("woT", [HH, H], BF16, kind="ExternalInput").ap()
    hbT = nc.dram_tensor("hbT", [128, KH], F32, kind="ExternalInput").ap()
    out_e = nc.dram_tensor("out", [H // 2, N], BF16, kind="ExternalOutput").ap()

    with tile.TileContext(nc) as tc:
        with (
            tc.tile_pool(name="singles", bufs=1) as singles,
            tc.tile_pool(name="ps", bufs=1, space="PSUM") as ps,
            tc.tile_pool(name="pe", bufs=16) as pe_pool,
            tc.tile_pool(name="rsb", bufs=6) as r_pool,
            tc.tile_pool(name="osb", bufs=6) as osb_pool,
            tc.tile_pool(name="dram", bufs=1, space="DRAM") as dram,
        ):
            x_sb = singles.tile([128, KH, N], BF16)
            wq_sb = singles.tile([128, KH, HH], BF16)
            wk_sb = singles.tile([128, KH, HH], BF16)
            wv_sb = singles.tile([128, KH, HH], BF16)
            wo_sb = singles.tile([128, MT, H], BF16)
            hb_sb = singles.tile([128, KH], F32)
            qT_sb = singles.tile([128, MT, N], BF16)
            kT_sb = singles.tile([128, MT, N], BF16)
            v_sb = singles.tile([128, NJT, 8, DH], BF16)
            mT_sb = singles.tile([128, MT, N], BF16)
            ones_sb = singles.tile([128, DH], BF16)

            rs_in = []
            rs_out = []
            for c in range(NCK):
                t_in = dram.tile([H, 512], BF16, tag=f"rsin{c}", name=f"rsin{c}")
                t_out = dram.tile([H // 2, 512], BF16, tag=f"rsout{c}",
                                  name=f"rsout{c}")
                rs_in.append(t_in)
                rs_out.append(t_out)

            nc.vector.memset(ones_sb, 1.0)
            # three DMA issue queues so the critical inputs (x halves, wk,
            # wq, wv) all land within the first ~12us
            for k in range(KH):
                nc.gpsimd.dma_start(out=x_sb[:, k, 0:1024],
                                    in_=xT[k * 128:(k + 1) * 128, 0:1024])
            for k in range(KH):
                nc.gpsimd.dma_start(out=x_sb[:, k, 1024:2048],
                                    in_=xT[k * 128:(k + 1) * 128, 1024:2048])
            for k in range(KH):
                nc.sync.dma_start(out=wk_sb[:, k, :], in_=wkT[k * 128:(k + 1) * 128, :])
            for k in range(KH):
                nc.sync.dma_start(out=wq_sb[:, k, :], in_=wqT[k * 128:(k + 1) * 128, :])
            for k in range(KH):
                nc.scalar.dma_start(out=wv_sb[:, k, :], in_=wvT[k * 128:(k + 1) * 128, :])
            for m in range(MT):
                nc.scalar.dma_start(out=wo_sb[:, m, :], in_=woT[m * 128:(m + 1) * 128, :])
            nc.scalar.dma_start(out=hb_sb, in_=hbT)

            def kq_proj(w_sb, dst, m, ci):
                pp = ps.tile([128, 512], F32, tag="proj", bufs=2,
                             name=f"kq{m}_{ci}")
                for k in range(KH):
                    nc.tensor.matmul(
                        pp,
                        lhsT=w_sb[:, k, m * 128:(m + 1) * 128],
                        rhs=x_sb[:, k, ci * 512:(ci + 1) * 512],
                        start=(k == 0), stop=(k == KH - 1),
                    )
                nc.vector.tensor_copy(out=dst[:, m, ci * 512:(ci + 1) * 512], in_=pp)

            def kq_half(w_sb, dst, m, ci, half, box):
                # half a kq_proj (4 of 8 k-tiles) so filler slices stay
                # under the exp period; box carries the PSUM tile across
                if half == 0:
                    box[0] = ps.tile([128, 512], F32, tag="proj", bufs=2,
                                     name=f"kqh{m}_{ci}")
                pp = box[0]
                for k in range(half * 4, half * 4 + 4):
                    nc.tensor.matmul(
                        pp,
                        lhsT=w_sb[:, k, m * 128:(m + 1) * 128],
                        rhs=x_sb[:, k, ci * 512:(ci + 1) * 512],
                        start=(k == 0), stop=(k == KH - 1),
                    )
                if half == 1:
                    nc.vector.tensor_copy(
                        out=dst[:, m, ci * 512:(ci + 1) * 512], in_=pp
                    )

            def v_proj(jt):
                pp = ps.tile([128, 512], F32, tag="proj", bufs=2,
                             name=f"vp{jt}")
                for k in range(KH):
                    nc.tensor.matmul(
                        pp,
                        lhsT=x_sb[:, k, jt * 128:(jt + 1) * 128],
                        rhs=wv_sb[:, k, :],
                        start=(k == 0), stop=(k == KH - 1),
                    )
                nc.vector.tensor_copy(
                    out=v_sb[:, jt, :, :],
                    in_=pp.rearrange("p (h d) -> p h d", h=8),
                )

            def outproj_m(ci, m, q=None):
                pp = ps.tile([128, 512], F32, tag="proj", bufs=2,
                             name=f"pp{ci}_{m}")
                for kd in range(MT):
                    nc.tensor.matmul(
                        pp,
                        lhsT=wo_sb[:, kd, m * 128:(m + 1) * 128],
                        rhs=mT_sb[:, kd, ci * 512:(ci + 1) * 512],
                        start=(kd == 0), stop=(kd == MT - 1),
                    )
                ob = osb_pool.tile([128, 512], BF16, tag="ob")
                nc.vector.tensor_scalar_add(
                    out=ob, in0=pp, scalar1=hb_sb[:, m:m + 1]
                )
                (q or nc.sync).dma_start(
                    out=rs_in[ci][m * 128:(m + 1) * 128, :], in_=ob
                )
                if m == 7:
                    nc.gpsimd.collective_compute(
                        "ReduceScatter",
                        mybir.AluOpType.add,
                        replica_groups=RG,
                        ins=[rs_in[ci][:, :]],
                        outs=[rs_out[ci][:, :]],
                    )
                    (q or nc.sync).dma_start(
                        out=out_e[:, ci * 512:(ci + 1) * 512],
                        in_=rs_out[ci][:, :],
                    )

            def attention_ci(ci, fillers):
                units = [(pr, jt) for pr in range(4) for jt in range(NJT)]
                st_tiles = {}

                def emit_scores(u):
                    pr, jt = units[u]
                    st = ps.tile([128, 2, 512], F32, tag="st", bufs=2,
                                 name=f"st{ci}_{u}")
                    nc.tensor.matmul(
                        st[:, 0, :],
                        lhsT=kT_sb[0:64, pr, jt * 128:(jt + 1) * 128],
                        rhs=qT_sb[0:64, pr, ci * 512:(ci + 1) * 512],
                        start=True, stop=True,
                    )
                    nc.tensor.matmul(
                        st[:, 1, :],
                        lhsT=kT_sb[64:128, pr, jt * 128:(jt + 1) * 128],
                        rhs=qT_sb[64:128, pr, ci * 512:(ci + 1) * 512],
                        start=True, stop=True,
                    )
                    st_tiles[u] = st

                def emit_pv(po, pd, pe, pr, jt, first, last):
                    h0, h1 = 2 * pr, 2 * pr + 1
                    nc.tensor.matmul(
                        po[0:64, :], lhsT=v_sb[:, jt, h0, :], rhs=pe[:, 0, :],
                        start=first, stop=last, skip_group_check=True,
                    )
                    nc.tensor.matmul(
                        po[64:128, :], lhsT=v_sb[:, jt, h1, :], rhs=pe[:, 1, :],
                        start=first, stop=last, skip_group_check=True,
                    )
                    nc.tensor.matmul(
                        pd[0:64, :], lhsT=ones_sb, rhs=pe[:, 0, :],
                        start=first, stop=last, skip_group_check=True,
                    )
                    nc.tensor.matmul(
                        pd[64:128, :], lhsT=ones_sb, rhs=pe[:, 1, :],
                        start=first, stop=last, skip_group_check=True,
                    )

                emit_scores(0)
                emit_scores(1)
                po = pd = None
                for u, (pr, jt) in enumerate(units):
                    first, last = (jt == 0), (jt == NJT - 1)
                    if first:
                        po = ps.tile([128, 512], F32, tag="po", bufs=1,
                                     name=f"po{pr}")
                        pd = ps.tile([128, 512], F32, tag="pd", bufs=1,
                                     name=f"pd{pr}")
                    pe = pe_pool.tile([128, 2, 512], BF16, tag="pe")
                    nc.scalar.activation(out=pe, in_=st_tiles.pop(u),
                                         func=Exp, scale=SCALE)
                    if u + 2 < len(units):
                        emit_scores(u + 2)
                    for fn in fillers.get(u, ()):
                        fn()
                    emit_pv(po, pd, pe, pr, jt, first, last)
                    if last:
                        r = r_pool.tile([128, 512], F32, tag="r")
                        nc.vector.reciprocal_approx_fast(out=r, in_=pd)
                        nc.vector.tensor_mul(
                            out=mT_sb[:, pr, ci * 512:(ci + 1) * 512],
                            in0=po, in1=r,
                        )

            def F(fn, *a):
                return lambda: fn(*a)

            def sched(items):
                # items: list of (slot, closure); -> dict slot -> [closures]
                m = {}
                for u, fn in items:
                    m.setdefault(u, []).append(fn)
                return m

            # ---- prologue: just enough for the exp stream to start
            # (K m0 all chunks, Q/K m1 for chunk 0, V j0/j1)
            for ci in range(NCK):
                kq_proj(wk_sb, kT_sb, 0, ci)
            kq_proj(wq_sb, qT_sb, 0, 0)
            kq_proj(wk_sb, kT_sb, 1, 0)
            kq_proj(wq_sb, qT_sb, 1, 0)
            v_proj(0)
            v_proj(1)

            # ---- chunk 0 fillers: V just-in-time, remaining K/Q staged
            # ahead of each pr's score deadlines (ci0 is PE-bound, so
            # coarse 8-matmul fillers are fine here)
            it0 = [(jt - 2, F(v_proj, jt)) for jt in range(2, NJT)]
            it0 += [(14, F(kq_proj, wk_sb, kT_sb, 1, 1)),
                    (15, F(kq_proj, wk_sb, kT_sb, 1, 2)),
                    (16, F(kq_proj, wk_sb, kT_sb, 1, 3)),
                    (17, F(kq_proj, wk_sb, kT_sb, 2, 0)),
                    (18, F(kq_proj, wq_sb, qT_sb, 2, 0)),
                    (19, F(kq_proj, wk_sb, kT_sb, 2, 1)),
                    (20, F(kq_proj, wk_sb, kT_sb, 2, 2)),
                    (21, F(kq_proj, wk_sb, kT_sb, 2, 3)),
                    (22, F(kq_proj, wk_sb, kT_sb, 3, 0)),
                    (23, F(kq_proj, wq_sb, qT_sb, 3, 0)),
                    (24, F(kq_proj, wk_sb, kT_sb, 3, 1)),
                    (25, F(kq_proj, wk_sb, kT_sb, 3, 2)),
                    (26, F(kq_proj, wk_sb, kT_sb, 3, 3)),
                    (27, F(kq_proj, wq_sb, qT_sb, 0, 1))]
            attention_ci(0, sched(it0))

            # ---- chunks 1..3: Q m1-3 of this chunk early (4-matmul
            # halves, spaced so the exp stream never starves), prior
            # chunk's out-proj spread mid-chunk, next chunk's Q m0 late
            for ci in range(1, NCK):
                it = []
                for i, m in enumerate((1, 2, 3)):
                    box = [None]
                    it.append((4 * i + 1, F(kq_half, wq_sb, qT_sb, m, ci, 0, box)))
                    it.append((4 * i + 3, F(kq_half, wq_sb, qT_sb, m, ci, 1, box)))
                for m in range(8):
                    it.append((14 + 3 * m, F(outproj_m, ci - 1, m)))
                if ci + 1 < NCK:
                    box = [None]
                    it.append((40, F(kq_half, wq_sb, qT_sb, 0, ci + 1, 0, box)))
                    it.append((42, F(kq_half, wq_sb, qT_sb, 0, ci + 1, 1, box)))
                attention_ci(ci, sched(it))
            for m in range(8):
                outproj_m(NCK - 1, m, q=nc.scalar)

    nc.compile()
    return nc


def _get_nc():
    if "nc" not in _cache:
        _cache["nc"] = _build()
    return _cache["nc"]


def _shard_inputs(x, w_qkv, w_out, b_out):
    bf16 = ml_dtypes.bfloat16
    in_maps = []
    for c in range(NCORES):
        b, hh = c // 2, c % 2
        r0 = hh * HH
        hbT = (0.5 * b_out).astype(np.float32).reshape(KH, 128).T
        in_maps.append({
            "xT": np.ascontiguousarray(x[b].T).astype(bf16),
            "wqT": np.ascontiguousarray(w_qkv[r0:r0 + HH, :].T).astype(bf16),
            "wkT": np.ascontiguousarray(w_qkv[H + r0:H + r0 + HH, :].T).astype(bf16),
            "wvT": np.ascontiguousarray(w_qkv[2 * H + r0:2 * H + r0 + HH, :].T).astype(bf16),
            "woT": np.ascontiguousarray(w_out[:, r0:r0 + HH].T).astype(bf16),
            "hbT": np.ascontiguousarray(hbT),
        })
    return in_maps


def _assemble(results):
    out = np.empty((B, N, H), dtype=np.float32)
    for b in range(B):
        lo = np.asarray(results[2 * b]["out"]).astype(np.float32)
        hi = np.asarray(results[2 * b + 1]["out"]).astype(np.float32)
        out[b][:, :512] = lo.T
        out[b][:, 512:] = hi.T
    return out


def run_sharded(x, w_qkv, w_out, b_out, trace=False):
    nc = _get_nc()
    in_maps = _shard_inputs(x, w_qkv, w_out, b_out)
    res = run_bass_kernel_spmd(nc, in_maps, core_ids=list(range(NCORES)),
                               trace=trace)
    return _assemble(res.results), res


def kernel(x, w_qkv, w_out, b_out):
    x = np.asarray(x, dtype=np.float32)
    w_qkv = np.asarray(w_qkv, dtype=np.float32)
    w_out = np.asarray(w_out, dtype=np.float32)
    b_out = np.asarray(b_out, dtype=np.float32)
    out, _ = run_sharded(x, w_qkv, w_out, b_out, trace=False)
    return out
